# revision 1
# baseline (speedup 1.0000x reference)
"""Trainium2 Bass kernel for the 32-iteration 3x3 survival automaton.

Problem: x is a 4096x4096 binary fp32 grid. 32 iterations of:
    keep cell iff its 8-neighbor live count > 3  (zero 'SAME' padding)
Output: scalar sum(x) - sum(y_final).

Strategy (8 NeuronCores, SPMD, zero inter-core communication):
  - Row-shard: core c owns rows [512c, 512c+512) and loads them plus a
    32-row halo per side; the halo is consumed one row per iteration, so
    after 32 iterations the owned rows are exact with no core-to-core
    traffic. One guard row/col of zeros emulates the 'SAME' zero padding
    (dead cells stay dead, so guards self-maintain).
  - Per-core slab: 578 rows x 4098 cols bf16, five 128-partition row tiles
    (stride 126, 2-row overlap; seam rows refreshed by tiny DMAs each iter).
  - Update algebra: with B[c] = y[c-1] + y[c+1] (VectorE shifted add),
        y_new = step( Tri@B + (Tri + 16 I)@y - 20.5 )
    Tri = tridiagonal ones band (vertical 3-tap conv as TensorE matmul);
    the 16*center fold makes one threshold express "alive AND >3 neighbors".
  - Threshold from PSUM: ScalarE Sigmoid(120*(s-20.5)) (saturates to exact
    1.0 / ~1e-26) and VectorE is_gt (exact), split for engine balance,
    processed 4 PSUM banks per instruction to amortize op overhead.
  - TensorE: matmuls grouped per stationary with ldweights=False on the
    tail of each group so the PE array keeps the loaded weights and
    back-to-back matmuls pipeline instead of reloading per-MM.
  - Final reduction: masked ones-vector matmuls accumulate column sums of
    owned rows into one PSUM bank; VectorE reduces to a scalar per core.
    Host sums 8 partials and subtracts from sum(x).
"""

import sys

if '/opt/trn_rl_repo' not in sys.path:
    sys.path.insert(0, '/opt/trn_rl_repo')

from contextlib import ExitStack, contextmanager

import ml_dtypes
import numpy as np

import concourse.bass as bass
import concourse.tile as tile
from concourse import bacc, mybir
from concourse.bass_utils import run_bass_kernel_spmd

# ---------------------------------------------------------------- geometry
H = W = 4096
NCORES = 8
OWN = H // NCORES            # 512 rows owned per core
HALO = 32                    # rows of redundant compute per side
SLAB_R = OWN + 2 * HALO + 2  # 578 (incl. 1 guard row each side)
SLAB_C = W + 2               # 4098 (incl. 1 guard col each side)
NT = 5                       # SBUF row-tiles per slab
KSH = 4                      # seam shrink depth: refresh every KSH iters
STRIDE = 128 - 2 * KSH       # 120 (8-row overlap between tiles)
OFF = [t * STRIDE for t in range(NT)]              # 0,120,240,360,480
RT = [min(128, SLAB_R - o) for o in OFF]           # 128,128,128,128,98
PSW = 1024                   # threshold granularity: 2 PSUM banks
NPS = W // PSW               # 4 psum units per row-tile
MMW = 512                    # matmul output free size (1 PSUM bank)
MPU = PSW // MMW             # matmuls per psum unit per stationary

# threshold split: per tile, units handled by ScalarE (sigmoid); the rest
# go to VectorE (exact is_gt). Tuned for engine balance.
ACT_UNITS = [4, 3, 3, 3, 3]

F32 = mybir.dt.float32
BF16 = mybir.dt.bfloat16


@contextmanager
def _no_ldweights():
    """Emit InstMatmult with ldweights=False: reuse the PE array's currently
    loaded stationary instead of reloading per matmul."""
    orig = mybir.InstMatmult

    def mk(*a, **kw):
        kw['ldweights'] = False
        return orig(*a, **kw)

    mybir.InstMatmult = mk
    try:
        yield
    finally:
        mybir.InstMatmult = orig


def _ldw_sig(inst):
    """Signature of the stationary operand an InstLdweights loads."""
    ap = inst.ins[0]
    return (getattr(ap, 'memref', None), getattr(ap, 'offset', None),
            str(getattr(ap, 'ap', None)), str(inst.tile_position),
            str(inst.tile_size), str(getattr(inst, 'perf_mode', None)),
            str(getattr(inst, 'is_transpose', None)))


def _dedup_ldweights(nc):
    """Remove InstLdweights that reload the stationary already in the PE
    array (same weights AP, only non-loading Matmults in between). Waits on
    a removed load are pushed onto the next PE instruction; loads carrying
    semaphore updates are kept."""
    removed = 0
    for f in nc.m.functions:
        for blk in f.blocks:
            cur = None
            out = []
            pending_waits = []
            for inst in blk.instructions:
                if isinstance(inst, mybir.InstLdweights):
                    sig = _ldw_sig(inst)
                    si = inst.sync_info
                    has_upd = si is not None and len(si.on_update) > 0
                    if sig == cur and not has_upd:
                        if si is not None and len(si.on_wait) > 0:
                            pending_waits.extend(si.on_wait)
                        removed += 1
                        continue
                    cur = sig
                elif isinstance(inst, mybir.InstMatmult):
                    if inst.is_transpose or getattr(inst, 'ldweights', None) is not False:
                        cur = None
                elif type(inst).__name__ == 'InstMatmultMx':
                    cur = None
                if pending_waits and isinstance(
                        inst, (mybir.InstLdweights, mybir.InstMatmult)):
                    si = inst.sync_info
                    if si is None:
                        inst.sync_info = mybir.SyncInfo(
                            on_wait=list(pending_waits), on_update=[])
                    else:
                        si.on_wait = list(si.on_wait) + pending_waits
                    pending_waits = []
                out.append(inst)
            assert not pending_waits
            if len(out) != len(blk.instructions):
                blk.instructions[:] = out
    return removed


def _build(iters: int):
    nc = bacc.Bacc("TRN2", target_bir_lowering=False, debug=False)
    x_d = nc.dram_tensor("x", [SLAB_R, SLAB_C], BF16, kind="ExternalInput").ap()
    tri_d = nc.dram_tensor("tri", [128, 128], BF16, kind="ExternalInput").ap()
    m16_d = nc.dram_tensor("m16", [128, 128], BF16, kind="ExternalInput").ap()
    rmask_d = nc.dram_tensor("rmask", [NT, 128], F32, kind="ExternalInput").ap()
    out_d = nc.dram_tensor("ysum", [1, 1], F32, kind="ExternalOutput").ap()

    add = mybir.AluOpType.add

    with tile.TileContext(nc) as tc, ExitStack() as ctx:
        const_pool = ctx.enter_context(tc.tile_pool(name="const", bufs=1))
        ypool = ctx.enter_context(tc.tile_pool(name="y", bufs=1))
        bpool = ctx.enter_context(tc.tile_pool(name="b", bufs=1))

        tri_sb = const_pool.tile([128, 128], BF16, tag="tri")
        nc.sync.dma_start(tri_sb[:], tri_d[:])
        m16_sb = const_pool.tile([128, 128], BF16, tag="m16")
        nc.sync.dma_start(m16_sb[:], m16_d[:])
        rmask_sb = []
        for t in range(NT):
            rm = const_pool.tile([128, 1], F32, tag=f"rmask{t}", name=f"rmask{t}")
            nc.sync.dma_start(rm[:], rmask_d[t:t + 1, :])
            rmask_sb.append(rm)
        bias_sb = const_pool.tile([128, 1], F32, tag="biasc", name="biasc")
        nc.gpsimd.memset(bias_sb[:], -2460.0)

        y_sb = [ypool.tile([RT[t], SLAB_C], BF16, tag=f"y{t}", name=f"y{t}")
                for t in range(NT)]
        b_sb = [bpool.tile([RT[t], W], BF16, tag=f"b{t}", name=f"b{t}")
                for t in range(NT)]

        # load (host already converted to bf16)
        for t in range(NT):
            nc.sync.dma_start(y_sb[t][:], x_d[OFF[t]:OFF[t] + RT[t], :])

        def emit_b(t):
            nc.vector.tensor_tensor(
                b_sb[t][:], y_sb[t][:, 0:W], y_sb[t][:, 2:W + 2], op=add)

        def emit_seam(t):
            # refresh the 2*KSH-row overlap between tiles t and t+1 (each
            # tile's outer KSH rows go stale over KSH iterations)
            nc.sync.dma_start(y_sb[t][128 - KSH:128, :],
                              y_sb[t + 1][KSH:2 * KSH, :])
            nc.sync.dma_start(y_sb[t + 1][0:KSH, :],
                              y_sb[t][STRIDE:STRIDE + KSH, :])

        acc_sb = [[const_pool.tile([128, 1], F32, tag=f"acc{t}_{u}",
                                   name=f"acc{t}_{u}") for u in range(NPS)]
                  for t in range(NT)]

        def emit_mms_thresholds(psum_pool, it, t, accum=False):
            r = RT[t]
            psums = [psum_pool.tile([r, PSW], F32, tag="ps",
                                    name=f"ps_{it}_{t}_{u}")
                     for u in range(NPS)]
            for s, (w_sb, coff) in enumerate([(tri_sb, 0), (m16_sb, 1)]):
                first = True
                for u in range(NPS):
                    for h in range(MPU):
                        c0 = u * PSW + h * MMW
                        src = b_sb[t] if s == 0 else y_sb[t]
                        args = (psums[u][:, h * MMW:(h + 1) * MMW],
                                w_sb[0:r, 0:r],
                                src[:, coff + c0:coff + c0 + MMW])
                        kw = dict(start=(s == 0), stop=(s == 1))
                        if first:
                            nc.tensor.matmul(*args, **kw)
                            first = False
                        else:
                            with _no_ldweights():
                                nc.tensor.matmul(*args, **kw)
            for u in range(NPS):
                dst = y_sb[t][:, 1 + u * PSW:1 + (u + 1) * PSW]
                aout = acc_sb[t][u][0:r, 0:1] if accum else None
                if u < ACT_UNITS[t]:
                    nc.scalar.activation(
                        dst, psums[u][:],
                        mybir.ActivationFunctionType.Sigmoid,
                        bias=bias_sb[0:r, 0:1], scale=120.0,
                        accum_out=aout)
                else:
                    if accum:
                        nc.vector.tensor_scalar(
                            dst, psums[u][:], 20.5, 0.0,
                            op0=mybir.AluOpType.is_gt,
                            op1=mybir.AluOpType.add, accum_out=aout)
                    else:
                        nc.vector.tensor_scalar(
                            dst, psums[u][:], 20.5, None,
                            op0=mybir.AluOpType.is_gt)

        # Software-pipelined wavefront with k=2 seam shrinkage: tiles
        # overlap by 4 rows, so seams need refreshing only every 2nd
        # iteration. On non-refresh boundaries a tile's next-iteration
        # B-pass depends only on its own threshold and is emitted right
        # after it -- TensorE rolls across the iteration boundary with no
        # bubble (and no HAM re-throttle). On refresh boundaries, seams are
        # refreshed as soon as both neighbor tiles are thresholded.
        with tc.tile_pool(name="ps", bufs=4, space="PSUM") as psum_pool:
            for t in range(NT):
                emit_b(t)
            for it in range(iters):
                last = it == iters - 1
                refresh = (it % KSH == KSH - 1) and not last
                for t in range(NT):
                    emit_mms_thresholds(psum_pool, it, t, accum=last)
                    if last:
                        continue
                    if refresh:
                        if t >= 1:
                            emit_seam(t - 1)
                        if t >= 2:
                            emit_b(t - 2)
                    else:
                        emit_b(t)
                if not last and refresh:
                    emit_b(NT - 2)
                    emit_b(NT - 1)

        # masked dot of the per-row accumulators from the last iteration's
        # thresholds: ysum = sum_t rmask[t] . (row sums of tile t)
        with tc.tile_pool(name="sps", bufs=1, space="PSUM") as spsum_pool:
            sps = spsum_pool.tile([1, 1], F32, tag="sum", name="sps")
            n_mm = NT * NPS
            k = 0
            for t in range(NT):
                for u in range(NPS):
                    nc.tensor.matmul(
                        sps[:], rmask_sb[t][0:RT[t], 0:1],
                        acc_sb[t][u][0:RT[t], 0:1],
                        start=(k == 0), stop=(k == n_mm - 1))
                    k += 1
            ssb = const_pool.tile([1, 1], F32, tag="ssum", name="ssb")
            nc.vector.tensor_copy(ssb[:], sps[:])
            nc.sync.dma_start(out_d[:], ssb[:])

    _dedup_ldweights(nc)
    # After dedup, the "most recent ldweights" a matmul's extra waits would
    # be moved to can sit many matmuls earlier in the PE stream — waiting
    # there can deadlock against producers scheduled in between. Skip the
    # pass; generate_event_semaphores enforces the 1-wait constraint by
    # splitting waits into standalone event-sem instructions in place.
    nc.move_matmul_waits_to_ldweights = lambda: None
    nc.compile()
    return nc


def _consts():
    i = np.arange(128)
    tri = (np.abs(i[:, None] - i[None, :]) <= 1).astype(np.float32)
    m16 = tri + 16.0 * np.eye(128, dtype=np.float32)
    # valid-row masks for the final sum: slab rows [33, 545) are the owned
    # 512 rows; each row is summed from the tile where it is seam-valid
    # (interior partitions [1, 127) after the last iteration).
    rmask = np.zeros((NT, 128), np.float32)
    bounds = [(33, 124), (4, 124), (4, 124), (4, 124), (4, 65)]
    for t, (a, b) in enumerate(bounds):
        rmask[t, a:b] = 1.0
    assert sum(b - a for a, b in bounds) == OWN
    bf = ml_dtypes.bfloat16
    return tri.astype(bf), m16.astype(bf), rmask


def _slabs(x: np.ndarray):
    g = np.zeros((H + 2 * HALO + 2, SLAB_C), ml_dtypes.bfloat16)
    g[HALO + 1:HALO + 1 + H, 1:1 + W] = x  # 0/1 values: exact in bf16
    return [np.ascontiguousarray(g[c * OWN:c * OWN + SLAB_R])
            for c in range(NCORES)]


_CACHE = {}


def _get_nc(iters: int):
    if iters not in _CACHE:
        _CACHE[iters] = _build(iters)
    return _CACHE[iters]


def kernel(x: np.ndarray, convs) -> np.ndarray:
    iters = int(convs)
    x = np.asarray(x, np.float32)
    assert x.shape == (H, W)
    nc = _get_nc(iters)
    tri, m16, rmask = _consts()
    in_maps = [{"x": s, "tri": tri, "m16": m16, "rmask": rmask}
               for s in _slabs(x)]
    res = run_bass_kernel_spmd(nc, in_maps, core_ids=list(range(NCORES)))
    y_sum = sum(float(res.results[c]["ysum"][0, 0]) for c in range(NCORES))
    x_sum = float(x.astype(np.float64).sum())
    return np.float32(x_sum - y_sum)


if __name__ == "__main__":
    rng = np.random.default_rng(0)
    x = np.round(rng.random((H, W))).astype(np.float32)
    got = kernel(x, 32)
    from scipy import signal
    K = np.array([[1, 1, 1], [1, 0, 1], [1, 1, 1]], np.float32)
    y = x.copy()
    for _ in range(32):
        s = signal.convolve2d(y, K, mode='same')
        y = np.where(s > 3.0, y, 0).astype(np.float32)
    want = x.sum(dtype=np.float64) - y.sum(dtype=np.float64)
    print(f"got {got}, want {want}, rel {abs(got - want) / abs(want):.3e}")

